# revision 53
# baseline (speedup 1.0000x reference)
"""Trainium2 Bass kernel for GCFAgg-style block:
    q1 = x@W1.T+b1; q2 = x@W2.T+b2; r = x@WR.T+br
    out = (q1 @ q2.T) @ r        (per batch, no softmax)

Algebraic restructuring: with x_aug = [x | 1] and W*_aug = [W* | b*],
    out = x_aug @ P_full,  P_full = Khat @ G_aug @ Rhat
where Khat = W1_aug.T @ W2_aug, Rhat = WR_aug.T. The device computes only
the core G = x.T @ x; every term of P_full involving the augmented
row/col of G_aug is a host-computable rank-1 correction folded into one
K=2 matmul per P chunk (lhsT2/rhs2), and v = P_full[512] is computed
entirely on the host (three matvecs via associativity) and added to the
device result out = x @ P[0:512] after the gather.

Phase 1 runs in fp8e4m3 with DoubleRow perf mode (two 128-token planes
contracted per PE instruction — 2x the bf16 rate); G accumulates in f32
PSUM and only the upper block-triangle is computed (lower via PE
transposes). Phases 2/3 run in bf16 (fp8 there would breach the 2e-2
error gate). Output is stored bf16 and upcast on host. End-to-end
~9e-3 max-rel error vs the fp32 reference (gate 2e-2) — validated
against an exact-input numpy simulation of the quantization chain.

Sharding: batch dim B=8, one batch per NeuronCore (data parallel).
Self-contained: hardcodes shapes (x: [8, 4096, 512] f32).
"""
import sys

sys.path.insert(0, "/opt/trn_rl_repo")

import numpy as np
import ml_dtypes

import concourse.mybir as mybir
import concourse.tile as tile
from concourse import bacc
from concourse.bass_utils import run_bass_kernel_spmd
from concourse.tile_rust import add_dep_helper

B = 8          # batch -> one per core
N = 4096       # tokens per batch
D = 512        # model dim
GP = 640       # augmented dim 513 padded to 5*128 (khat col pad)
NT = N // 128  # 32 row tiles
NT2 = NT // 2  # 16 fp8 double-tiles for phase 1
N_CORES = 8

F32 = mybir.dt.float32
BF16 = mybir.dt.bfloat16
F8E4 = mybir.dt.float8e4
DR = mybir.MatmulPerfMode.DoubleRow
BF = ml_dtypes.bfloat16
F8 = ml_dtypes.float8_e4m3

_built = {}


def _build(key="v3"):
    if key in _built:
        return _built[key]

    nc = bacc.Bacc("TRN2", target_bir_lowering=False, debug=False,
                   num_devices=N_CORES)

    # one concatenated constant block -> one DMA, one semaphore:
    # cols [khat c*GP ...] x4 | [rhat c*D ...] x4 | ones 128
    CB = 4 * GP + 4 * D + 128
    xa8_d = nc.dram_tensor("xa8", (NT2 // 2, 128, 2, 2, D), F8E4,
                           kind="ExternalInput")
    xat_d = nc.dram_tensor("xat", (NT, 128, 4, 128), BF16,
                           kind="ExternalInput")
    cb_d = nc.dram_tensor("cb", (128, CB), BF16, kind="ExternalInput")
    ident_d = nc.dram_tensor("ident", (128, 128), BF16, kind="ExternalInput")
    aug2_d = nc.dram_tensor("aug2", (2, GP + D), BF16, kind="ExternalInput")
    out_d = nc.dram_tensor("out", (N, D), BF16, kind="ExternalOutput")

    with tile.TileContext(nc) as tc:
        with (
            tc.tile_pool(name="xa", bufs=10) as xa_pool,
            tc.tile_pool(name="xat", bufs=32) as xat_pool,
            tc.tile_pool(name="const", bufs=1) as const_pool,
            tc.tile_pool(name="gsb", bufs=1) as g_pool,
            tc.tile_pool(name="chain", bufs=1) as chain_pool,
            tc.tile_pool(name="outsb", bufs=6) as out_pool,
        ):
            cb_sb = const_pool.tile([128, CB], BF16, tag="cb")
            ident_sb = const_pool.tile([128, 128], BF16, tag="identsb")
            aug2_sb = const_pool.tile([2, GP + D], BF16, tag="aug2")
            # tiny aug2 first on the otherwise-idle scalar queue (feeds the
            # chain's leading K=2 fold); ident is queued after the first
            # xa half-tile below
            nc.scalar.dma_start(aug2_sb[:], aug2_d.ap()[:])

            # ---- phase 1: G = x^T @ x, fp8 DoubleRow over 16 double-tiles
            # (upper block-triangle only; xa8 streams on two DMA queues) ----
            g_sb = [g_pool.tile([128, D], BF16, tag=f"g{c}", name=f"g{c}")
                    for c in range(4)]
            with tc.tile_pool(name="psG", bufs=1, space="PSUM") as psG_pool:
                ps_ga = [psG_pool.tile([128, D - c * 128], F32, tag=f"ga{c}",
                                       name=f"ga{c}") for c in range(4)]
                # PE p-state warmup: junk matmuls on a memset tile keep the
                # PE continuously busy through the startup dead time so the
                # real phase-1 matmuls begin at full clock
                junk_sb = const_pool.tile([128, 128], BF16, tag="junk")
                nc.vector.memset(junk_sb[:], 0.25)
                ps_junk = psG_pool.tile([128, 128], F32, tag="psjunk")
                for _ in range(32):
                    nc.tensor.matmul(ps_junk[:], junk_sb[:], junk_sb[:],
                                     start=True, stop=True)
                gate_mms = []
                for pr in range(NT2 // 2):
                    # pair-sized DMAs (256 KB, 2-KB rows): half the
                    # completion semaphores the PE must wait on, and
                    # better DMA packet efficiency
                    xa_t = xa_pool.tile([128, 2, 2, D], F8E4, tag="xa")
                    if pr == 0:
                        # split the first pair by k-plane so the first
                        # matmul's data lands as early as possible
                        nc.sync.dma_start(xa_t[:, 0, :, :],
                                          xa8_d.ap()[pr, :, 0, :, :])
                        nc.sync.dma_start(xa_t[:, 1, :, :],
                                          xa8_d.ap()[pr, :, 1, :, :])
                        nc.scalar.dma_start(ident_sb[:], ident_d.ap()[:])
                    else:
                        eng = nc.gpsimd if pr % 2 == 1 else nc.sync
                        eng.dma_start(xa_t[:], xa8_d.ap()[pr])
                    for s in range(2):
                        t = 2 * pr + s
                        for c in range(4):
                            mm = nc.tensor.matmul(
                                ps_ga[c][:],
                                xa_t[:, :, s, c * 128:(c + 1) * 128],
                                xa_t[:, :, s, c * 128:D],
                                start=(t == 0), stop=(t == NT2 - 1),
                                perf_mode=DR,
                            )
                            if c == 3:
                                gate_mms.append(mm)
                # bulk chain constants on gpsimd in two pieces (rhat is
                # needed ~1us before khat), gated late in G so phase-1
                # bandwidth stays with the xa stream
                cbd1 = nc.gpsimd.dma_start(cb_sb[:, 4 * GP:CB],
                                           cb_d.ap()[:, 4 * GP:CB])
                add_dep_helper(cbd1.ins, gate_mms[9].ins,
                               reason="rhat block gated behind G t=9")
                cbd2 = nc.gpsimd.dma_start(cb_sb[:, 0:4 * GP],
                                           cb_d.ap()[:, 0:4 * GP])
                add_dep_helper(cbd2.ins, gate_mms[10].ins,
                               reason="khat block gated behind G t=10")
                # upper blocks into SBUF (cast f32 PSUM -> bf16), spread
                # across vector/scalar so casts run in parallel (gpsimd
                # cannot read PSUM on TRN2)
                def ps_copy(i, dst, src):
                    if i % 2 == 0:
                        nc.vector.tensor_copy(dst, src)
                    else:
                        nc.scalar.copy(dst, src)

                for c in range(4):
                    ps_copy(c, g_sb[c][:, c * 128:D], ps_ga[c][:])

            # ---- phase 2: P_full = Khat @ G_aug @ Rhat ----
            # K=2 aug fold opens each PSUM group; the G-symmetric-fill
            # transposes are interleaved with the chain: M1 runs j=3..0
            # because M1[j] only needs the transposed lower blocks (j,k>j)
            with tc.tile_pool(name="psC", bufs=1, space="PSUM") as psC_pool:
                m1_sb = [chain_pool.tile([128, D], BF16, tag=f"m1{c}",
                                         name=f"m1{c}") for c in range(4)]
                p_sb = [chain_pool.tile([128, D], BF16, tag=f"p{c}",
                                        name=f"p{c}") for c in range(4)]
                ps_p = [psC_pool.tile([128, D], F32, tag=f"pp{c}",
                                      name=f"pp{c}") for c in range(4)]
                ps_m1 = [psC_pool.tile([128, D], F32, tag="m1ps", bufs=2,
                                       name=f"m1ps{j}") for j in range(4)]

                tr_i = 0

                def emit_tr(c1, c2):
                    # lower block (c2,c1) = transpose of upper (c1,c2)
                    nonlocal tr_i
                    ps_tr = psC_pool.tile([128, 128], BF16, tag="tr",
                                          bufs=2, name=f"tr{c1}{c2}")
                    nc.tensor.transpose(
                        ps_tr[:],
                        g_sb[c1][:, c2 * 128:(c2 + 1) * 128],
                        ident_sb[:],
                    )
                    ps_copy(tr_i, g_sb[c2][:, c1 * 128:(c1 + 1) * 128],
                            ps_tr[:])
                    tr_i += 1

                def khat(j, lo, hi):
                    return cb_sb[:, j * GP + lo:j * GP + hi]

                def rhat(k):
                    return cb_sb[:, 4 * GP + k * D:4 * GP + (k + 1) * D]

                for i in range(4):
                    nc.tensor.matmul(
                        ps_p[i][:], aug2_sb[:, i * 128:(i + 1) * 128],
                        aug2_sb[:, GP:GP + D], start=True, stop=False,
                    )
                def emit_m1(j):
                    for k in range(4):
                        nc.tensor.matmul(
                            ps_m1[j][:], g_sb[k][:, j * 128:(j + 1) * 128],
                            rhat(k), start=(k == 0), stop=(k == 3),
                        )
                    ps_copy(j, m1_sb[j][:], ps_m1[j][:])

                def emit_pstep(j, last=False):
                    for i in range(4):
                        nc.tensor.matmul(
                            ps_p[i][:], khat(j, i * 128, (i + 1) * 128),
                            m1_sb[j][:], start=False, stop=last,
                        )
                        if last:
                            # cast each P chunk as soon as its group closes
                            ps_copy(i, p_sb[i][:], ps_p[i][:])

                emit_m1(3)
                emit_tr(2, 3)
                emit_m1(2)
                emit_pstep(3)
                emit_tr(1, 2)
                emit_tr(1, 3)
                emit_m1(1)
                emit_pstep(2)
                emit_tr(0, 1)
                emit_tr(0, 2)
                emit_tr(0, 3)
                emit_m1(0)
                emit_pstep(1)
                emit_pstep(0, last=True)


            # ---- phase 3: out = x @ P (v = P_full[512] is a pure
            # function of host-known data and is added on the host) ----
            with tc.tile_pool(name="psO", bufs=1, space="PSUM") as psO_pool:

                # all xat triggers emitted BEFORE the compute loop so no
                # prefetch trigger queues behind a blocked store trigger.
                # evens on gpsimd, odds on sync — NOT scalar: the scalar
                # ENGINE also executes the chain's PSUM copies, which must
                # not queue behind 16 trigger instructions. Release is
                # shaped behind G progress so warmup bandwidth stays with
                # the xa stream.
                xat_tiles = []
                for t in range(NT):
                    xat_t = xat_pool.tile([128, 4, 128], BF16, tag="xat")
                    eng = nc.gpsimd if t % 2 == 0 else nc.sync
                    gate = 5 + t // 4 if t % 2 == 0 else 10 + t // 8
                    xdma = eng.dma_start(xat_t[:], xat_d.ap()[t])
                    add_dep_helper(xdma.ins,
                                   gate_mms[min(NT2 - 1, gate)].ins,
                                   reason="xat prefetch shaped behind G")
                    xat_tiles.append(xat_t)

                for t in range(NT):
                    xat_t = xat_tiles[t]
                    ps = psO_pool.tile([128, D], F32, tag="out", bufs=6)
                    for c in range(4):
                        nc.tensor.matmul(
                            ps[:], xat_t[:, c, :], p_sb[c][:],
                            start=(c == 0), stop=(c == 3),
                        )
                    ot = out_pool.tile([128, D], BF16, tag="ot")
                    nc.vector.tensor_copy(ot[:], ps[:])
                    if t >= NT - 2:
                        # split the final stores across sync+scalar (NOT
                        # gpsimd, whose queue drain is the long pole in the
                        # epilogue) so the last data lands fast
                        half = D // 2
                        nc.sync.dma_start(
                            out_d.ap()[t * 128:(t + 1) * 128, 0:half],
                            ot[:, 0:half])
                        nc.scalar.dma_start(
                            out_d.ap()[t * 128:(t + 1) * 128, half:D],
                            ot[:, half:D])
                    else:
                        # late even stores go to scalar so the gpsimd
                        # queue's (long) epilogue drain starts earlier
                        if t % 2 == 1:
                            eng = nc.sync
                        elif t >= 24:
                            eng = nc.scalar
                        else:
                            eng = nc.gpsimd
                        eng.dma_start(out_d.ap()[t * 128:(t + 1) * 128, :],
                                      ot[:])

    nc.compile()
    _built[key] = nc
    return nc


def _prep_host(x, Wq1_w, Wq1_b, Wq2_w, Wq2_b, WR_w, WR_b):
    f = np.float32
    W1a = np.concatenate([Wq1_w, Wq1_b[:, None]], axis=1)   # [512, 513]
    W2a = np.concatenate([Wq2_w, Wq2_b[:, None]], axis=1)
    WRa = np.concatenate([WR_w, WR_b[:, None]], axis=1)

    Khat = (W1a.T.astype(np.float64) @ W2a.astype(np.float64))  # [513, 513]
    Rhat = WRa.T.astype(np.float64)                             # [513, 512]

    # concatenated constant block: khat row-chunks | rhat chunks | ones
    CB = 4 * GP + 4 * D + 128
    cb = np.zeros((128, CB), f)
    khatT = np.zeros((4, 128, GP), f)   # Khat^T core row-chunks, col-padded
    khatT[:, :, :513] = Khat.T[:512].reshape(4, 128, 513).astype(f)
    for c in range(4):
        cb[:, c * GP:(c + 1) * GP] = khatT[c]
        cb[:, 4 * GP + c * D:4 * GP + (c + 1) * D] = \
            Rhat[c * 128:(c + 1) * 128].astype(f)
    cb[0, 4 * GP + 4 * D:CB] = 1.0

    # augmented rank-1 folds (everything touching G_aug's row/col 512):
    #   P_full += u1 (x) rhat_row + khat_col (x) m1row
    sx = x.sum(axis=1, dtype=np.float64)                 # [B, 512]
    sxa = np.concatenate([sx, np.full((B, 1), float(N))], axis=1)  # [B, 513]
    m1row = sxa @ Rhat                                   # [B, 512]
    u1 = np.einsum('ij,bj->bi', Khat[:, :512], sx)       # [B, 513]

    aug2 = np.zeros((B, 2, GP + D), f)
    aug2[:, 0, :513] = u1.astype(f)
    aug2[:, 1, :513] = Khat[:, 512].astype(f)[None, :]
    aug2[:, 0, GP:GP + D] = Rhat[512].astype(f)[None, :]
    aug2[:, 1, GP:GP + D] = m1row.astype(f)

    # v = P_full[512] = Khat[512] @ G_aug @ Rhat is a pure function of
    # host-known data (three matvecs via associativity) — computed here
    # in f64 and added to the device output after the gather
    kvec = Khat[512]                                     # [513]
    w = np.einsum('bnd,d->bn', x.astype(np.float64), kvec[:512]) + kvec[512]
    u = np.concatenate([np.einsum('bnd,bn->bd', x.astype(np.float64), w),
                        w.sum(axis=1)[:, None]], axis=1)  # [B, 513]
    v_host = (u @ Rhat).astype(f)                        # [B, 512]

    # fp8 phase-1 layout, pair-batched:
    # xa8[b, pr, p, h, s, d] = x[b, (2*pr+s)*256 + h*128 + p, d]
    xa8 = np.ascontiguousarray(
        x.astype(F8).reshape(B, NT2 // 2, 2, 2, 128, D)
         .transpose(0, 1, 4, 3, 2, 5))

    # xat[b, t, p, c, j] = x[b, t*128+j, c*128+p] — per-tile [128, 4, 128]
    # lhsT blocks of x^T
    xat = np.ascontiguousarray(
        x.transpose(0, 2, 1)                     # [B, 512, 4096]
         .reshape(B, 4, 128, NT, 128)            # [B, c, p, t, j]
         .transpose(0, 3, 2, 1, 4)               # [B, t, p, c, j]
    ).astype(BF)

    cb_bf = cb.astype(BF)
    ident = np.eye(128, dtype=f).astype(BF)
    in_maps = [
        {"xa8": xa8[b], "xat": xat[b], "cb": cb_bf, "ident": ident,
         "aug2": aug2[b].astype(BF)}
        for b in range(B)
    ]
    return in_maps, v_host


def kernel(x, Wq1_w, Wq1_b, Wq2_w, Wq2_b, WR_w, WR_b):
    x = np.asarray(x, dtype=np.float32)
    args = [np.asarray(a, dtype=np.float32)
            for a in (Wq1_w, Wq1_b, Wq2_w, Wq2_b, WR_w, WR_b)]
    in_maps, v_host = _prep_host(x, *args)

    nc = _build()
    # the axon-tunneled device occasionally starts in a wedged state
    # (NRT_EXEC_UNIT_UNRECOVERABLE) and recovers on the next attempt
    last_err = None
    for attempt in range(3):
        try:
            res = run_bass_kernel_spmd(nc, in_maps, core_ids=list(range(N_CORES)))
            break
        except Exception as e:  # noqa: BLE001
            last_err = e
            import time as _time
            _time.sleep(2.0)
            try:
                import jax
                jax.clear_caches()
            except Exception:
                pass
    else:
        raise last_err
    return np.stack([res.results[b]["out"].astype(np.float32) + v_host[b]
                     for b in range(B)])


# revision 55
# speedup vs baseline: 1.0235x; 1.0235x over previous
"""Trainium2 Bass kernel for GCFAgg-style block:
    q1 = x@W1.T+b1; q2 = x@W2.T+b2; r = x@WR.T+br
    out = (q1 @ q2.T) @ r        (per batch, no softmax)

Algebraic restructuring: with x_aug = [x | 1] and W*_aug = [W* | b*],
    out = x_aug @ P_full,  P_full = Khat @ G_aug @ Rhat
where Khat = W1_aug.T @ W2_aug, Rhat = WR_aug.T. The device computes only
the core G = x.T @ x; every term of P_full involving the augmented
row/col of G_aug is a host-computable rank-1 correction folded into one
K=2 matmul per P chunk (lhsT2/rhs2), and v = P_full[512] is computed
entirely on the host (three matvecs via associativity) and added to the
device result out = x @ P[0:512] after the gather.

Phase 1 runs in fp8e4m3 with DoubleRow perf mode (two 128-token planes
contracted per PE instruction — 2x the bf16 rate); G accumulates in f32
PSUM and only the upper block-triangle is computed (lower via PE
transposes). Phases 2/3 run in bf16 (fp8 there would breach the 2e-2
error gate). Output is stored bf16 and upcast on host. End-to-end
~9e-3 max-rel error vs the fp32 reference (gate 2e-2) — validated
against an exact-input numpy simulation of the quantization chain.

Sharding: batch dim B=8, one batch per NeuronCore (data parallel).
Self-contained: hardcodes shapes (x: [8, 4096, 512] f32).
"""
import sys

sys.path.insert(0, "/opt/trn_rl_repo")

import numpy as np
import ml_dtypes

import concourse.mybir as mybir
import concourse.tile as tile
from concourse import bacc
from concourse.bass_utils import run_bass_kernel_spmd
from concourse.tile_rust import add_dep_helper

B = 8          # batch -> one per core
N = 4096       # tokens per batch
D = 512        # model dim
GP = 640       # augmented dim 513 padded to 5*128 (khat col pad)
NT = N // 128  # 32 row tiles
NT2 = NT // 2  # 16 fp8 double-tiles for phase 1
N_CORES = 8

F32 = mybir.dt.float32
BF16 = mybir.dt.bfloat16
F8E4 = mybir.dt.float8e4
DR = mybir.MatmulPerfMode.DoubleRow
BF = ml_dtypes.bfloat16
F8 = ml_dtypes.float8_e4m3

_built = {}


def _build(key="v3"):
    if key in _built:
        return _built[key]

    nc = bacc.Bacc("TRN2", target_bir_lowering=False, debug=False,
                   num_devices=N_CORES)

    # one concatenated constant block -> one DMA, one semaphore:
    # cols [khat c*GP ...] x4 | [rhat c*D ...] x4 | ones 128
    CB = 4 * GP + 4 * D + 128
    xa8_d = nc.dram_tensor("xa8", (NT2 // 2, 128, 2, 2, D), F8E4,
                           kind="ExternalInput")
    xat_d = nc.dram_tensor("xat", (NT, 128, 4, 128), BF16,
                           kind="ExternalInput")
    cb_d = nc.dram_tensor("cb", (128, CB), BF16, kind="ExternalInput")
    ident_d = nc.dram_tensor("ident", (128, 128), BF16, kind="ExternalInput")
    aug2_d = nc.dram_tensor("aug2", (2, GP + D), BF16, kind="ExternalInput")
    out_d = nc.dram_tensor("out", (N, D), BF16, kind="ExternalOutput")

    with tile.TileContext(nc) as tc:
        with (
            tc.tile_pool(name="xa", bufs=10) as xa_pool,
            tc.tile_pool(name="xat", bufs=32) as xat_pool,
            tc.tile_pool(name="const", bufs=1) as const_pool,
            tc.tile_pool(name="gsb", bufs=1) as g_pool,
            tc.tile_pool(name="chain", bufs=1) as chain_pool,
            tc.tile_pool(name="outsb", bufs=6) as out_pool,
        ):
            cb_sb = const_pool.tile([128, CB], BF16, tag="cb")
            ident_sb = const_pool.tile([128, 128], BF16, tag="identsb")
            aug2_sb = const_pool.tile([2, GP + D], BF16, tag="aug2")
            # tiny aug2 first on the otherwise-idle scalar queue (feeds the
            # chain's leading K=2 fold); ident is queued after the first
            # xa half-tile below
            nc.scalar.dma_start(aug2_sb[:], aug2_d.ap()[:])

            # ---- phase 1: G = x^T @ x, fp8 DoubleRow over 16 double-tiles
            # (upper block-triangle only; xa8 streams on two DMA queues) ----
            g_sb = [g_pool.tile([128, D], BF16, tag=f"g{c}", name=f"g{c}")
                    for c in range(4)]
            with tc.tile_pool(name="psG", bufs=1, space="PSUM") as psG_pool:
                ps_ga = [psG_pool.tile([128, D - c * 128], F32, tag=f"ga{c}",
                                       name=f"ga{c}") for c in range(4)]
                # PE p-state warmup: junk matmuls on a memset tile keep the
                # PE continuously busy through the startup dead time so the
                # real phase-1 matmuls begin at full clock
                junk_sb = const_pool.tile([128, 128], BF16, tag="junk")
                nc.vector.memset(junk_sb[:], 0.25)
                ps_junk = psG_pool.tile([128, 128], F32, tag="psjunk")
                for _ in range(32):
                    nc.tensor.matmul(ps_junk[:], junk_sb[:], junk_sb[:],
                                     start=True, stop=True)
                gate_mms = []
                for pr in range(NT2 // 2):
                    # pair-sized DMAs (256 KB, 2-KB rows): half the
                    # completion semaphores the PE must wait on and better
                    # packet efficiency. Pair 0 is split three ways so the
                    # first matmul's operands land as early as possible:
                    # two 64-KB halves of sub-tile 0 on sync, sub-tile 1
                    # in parallel on gpsimd.
                    xa_t = xa_pool.tile([128, 2, 2, D], F8E4, tag="xa")
                    if pr == 0:
                        nc.sync.dma_start(xa_t[:, 0, 0, :],
                                          xa8_d.ap()[pr, :, 0, 0, :])
                        nc.sync.dma_start(xa_t[:, 1, 0, :],
                                          xa8_d.ap()[pr, :, 1, 0, :])
                        nc.gpsimd.dma_start(xa_t[:, :, 1, :],
                                            xa8_d.ap()[pr, :, :, 1, :])
                        nc.scalar.dma_start(ident_sb[:], ident_d.ap()[:])
                    else:
                        eng = nc.gpsimd if pr % 2 == 1 else nc.sync
                        eng.dma_start(xa_t[:], xa8_d.ap()[pr])
                    for s in range(2):
                        t = 2 * pr + s
                        for c in range(4):
                            mm = nc.tensor.matmul(
                                ps_ga[c][:],
                                xa_t[:, :, s, c * 128:(c + 1) * 128],
                                xa_t[:, :, s, c * 128:D],
                                start=(t == 0), stop=(t == NT2 - 1),
                                perf_mode=DR,
                            )
                            if c == 3:
                                gate_mms.append(mm)
                # bulk chain constants on gpsimd in two pieces (rhat is
                # needed ~1us before khat), gated late in G so phase-1
                # bandwidth stays with the xa stream
                cbd1 = nc.gpsimd.dma_start(cb_sb[:, 4 * GP:CB],
                                           cb_d.ap()[:, 4 * GP:CB])
                add_dep_helper(cbd1.ins, gate_mms[9].ins,
                               reason="rhat block gated behind G t=9")
                cbd2 = nc.gpsimd.dma_start(cb_sb[:, 0:4 * GP],
                                           cb_d.ap()[:, 0:4 * GP])
                add_dep_helper(cbd2.ins, gate_mms[10].ins,
                               reason="khat block gated behind G t=10")
                # upper blocks into SBUF (cast f32 PSUM -> bf16), spread
                # across vector/scalar so casts run in parallel (gpsimd
                # cannot read PSUM on TRN2)
                def ps_copy(i, dst, src):
                    if i % 2 == 0:
                        nc.vector.tensor_copy(dst, src)
                    else:
                        nc.scalar.copy(dst, src)

                for c in range(4):
                    ps_copy(c, g_sb[c][:, c * 128:D], ps_ga[c][:])

            # ---- phase 2: P_full = Khat @ G_aug @ Rhat ----
            # K=2 aug fold opens each PSUM group; the G-symmetric-fill
            # transposes are interleaved with the chain: M1 runs j=3..0
            # because M1[j] only needs the transposed lower blocks (j,k>j)
            with tc.tile_pool(name="psC", bufs=1, space="PSUM") as psC_pool:
                m1_sb = [chain_pool.tile([128, D], BF16, tag=f"m1{c}",
                                         name=f"m1{c}") for c in range(4)]
                p_sb = [chain_pool.tile([128, D], BF16, tag=f"p{c}",
                                        name=f"p{c}") for c in range(4)]
                ps_p = [psC_pool.tile([128, D], F32, tag=f"pp{c}",
                                      name=f"pp{c}") for c in range(4)]
                ps_m1 = [psC_pool.tile([128, D], F32, tag="m1ps", bufs=2,
                                       name=f"m1ps{j}") for j in range(4)]

                tr_i = 0

                def emit_tr(c1, c2):
                    # lower block (c2,c1) = transpose of upper (c1,c2)
                    nonlocal tr_i
                    ps_tr = psC_pool.tile([128, 128], BF16, tag="tr",
                                          bufs=2, name=f"tr{c1}{c2}")
                    nc.tensor.transpose(
                        ps_tr[:],
                        g_sb[c1][:, c2 * 128:(c2 + 1) * 128],
                        ident_sb[:],
                    )
                    ps_copy(tr_i, g_sb[c2][:, c1 * 128:(c1 + 1) * 128],
                            ps_tr[:])
                    tr_i += 1

                def khat(j, lo, hi):
                    return cb_sb[:, j * GP + lo:j * GP + hi]

                def rhat(k):
                    return cb_sb[:, 4 * GP + k * D:4 * GP + (k + 1) * D]

                for i in range(4):
                    nc.tensor.matmul(
                        ps_p[i][:], aug2_sb[:, i * 128:(i + 1) * 128],
                        aug2_sb[:, GP:GP + D], start=True, stop=False,
                    )
                def emit_m1(j):
                    for k in range(4):
                        nc.tensor.matmul(
                            ps_m1[j][:], g_sb[k][:, j * 128:(j + 1) * 128],
                            rhat(k), start=(k == 0), stop=(k == 3),
                        )
                    ps_copy(j, m1_sb[j][:], ps_m1[j][:])

                def emit_pstep(j, last=False):
                    for i in range(4):
                        nc.tensor.matmul(
                            ps_p[i][:], khat(j, i * 128, (i + 1) * 128),
                            m1_sb[j][:], start=False, stop=last,
                        )
                        if last:
                            # cast each P chunk as soon as its group closes
                            ps_copy(i, p_sb[i][:], ps_p[i][:])

                emit_m1(3)
                emit_tr(2, 3)
                emit_m1(2)
                emit_pstep(3)
                emit_tr(1, 2)
                emit_tr(1, 3)
                emit_m1(1)
                emit_pstep(2)
                emit_tr(0, 1)
                emit_tr(0, 2)
                emit_tr(0, 3)
                emit_m1(0)
                emit_pstep(1)
                emit_pstep(0, last=True)


            # ---- phase 3: out = x @ P (v = P_full[512] is a pure
            # function of host-known data and is added on the host) ----
            with tc.tile_pool(name="psO", bufs=1, space="PSUM") as psO_pool:

                # all xat triggers emitted BEFORE the compute loop so no
                # prefetch trigger queues behind a blocked store trigger.
                # evens on gpsimd, odds on sync — NOT scalar: the scalar
                # ENGINE also executes the chain's PSUM copies, which must
                # not queue behind 16 trigger instructions. Release is
                # shaped behind G progress so warmup bandwidth stays with
                # the xa stream.
                xat_tiles = []
                for t in range(NT):
                    xat_t = xat_pool.tile([128, 4, 128], BF16, tag="xat")
                    eng = nc.gpsimd if t % 2 == 0 else nc.sync
                    gate = 5 + t // 4 if t % 2 == 0 else 10 + t // 8
                    xdma = eng.dma_start(xat_t[:], xat_d.ap()[t])
                    add_dep_helper(xdma.ins,
                                   gate_mms[min(NT2 - 1, gate)].ins,
                                   reason="xat prefetch shaped behind G")
                    xat_tiles.append(xat_t)

                for t in range(NT):
                    xat_t = xat_tiles[t]
                    ps = psO_pool.tile([128, D], F32, tag="out", bufs=6)
                    for c in range(4):
                        nc.tensor.matmul(
                            ps[:], xat_t[:, c, :], p_sb[c][:],
                            start=(c == 0), stop=(c == 3),
                        )
                    ot = out_pool.tile([128, D], BF16, tag="ot")
                    nc.vector.tensor_copy(ot[:], ps[:])
                    if t >= NT - 2:
                        # split the final stores across sync+scalar (NOT
                        # gpsimd, whose queue drain is the long pole in the
                        # epilogue) so the last data lands fast
                        half = D // 2
                        nc.sync.dma_start(
                            out_d.ap()[t * 128:(t + 1) * 128, 0:half],
                            ot[:, 0:half])
                        nc.scalar.dma_start(
                            out_d.ap()[t * 128:(t + 1) * 128, half:D],
                            ot[:, half:D])
                    else:
                        # late even stores go to scalar so the gpsimd
                        # queue's (long) epilogue drain starts earlier
                        if t % 2 == 1:
                            eng = nc.sync
                        elif t >= 24:
                            eng = nc.scalar
                        else:
                            eng = nc.gpsimd
                        eng.dma_start(out_d.ap()[t * 128:(t + 1) * 128, :],
                                      ot[:])

    nc.compile()
    _built[key] = nc
    return nc


def _prep_host(x, Wq1_w, Wq1_b, Wq2_w, Wq2_b, WR_w, WR_b):
    f = np.float32
    W1a = np.concatenate([Wq1_w, Wq1_b[:, None]], axis=1)   # [512, 513]
    W2a = np.concatenate([Wq2_w, Wq2_b[:, None]], axis=1)
    WRa = np.concatenate([WR_w, WR_b[:, None]], axis=1)

    Khat = (W1a.T.astype(np.float64) @ W2a.astype(np.float64))  # [513, 513]
    Rhat = WRa.T.astype(np.float64)                             # [513, 512]

    # concatenated constant block: khat row-chunks | rhat chunks | ones
    CB = 4 * GP + 4 * D + 128
    cb = np.zeros((128, CB), f)
    khatT = np.zeros((4, 128, GP), f)   # Khat^T core row-chunks, col-padded
    khatT[:, :, :513] = Khat.T[:512].reshape(4, 128, 513).astype(f)
    for c in range(4):
        cb[:, c * GP:(c + 1) * GP] = khatT[c]
        cb[:, 4 * GP + c * D:4 * GP + (c + 1) * D] = \
            Rhat[c * 128:(c + 1) * 128].astype(f)
    cb[0, 4 * GP + 4 * D:CB] = 1.0

    # augmented rank-1 folds (everything touching G_aug's row/col 512):
    #   P_full += u1 (x) rhat_row + khat_col (x) m1row
    sx = x.sum(axis=1, dtype=np.float64)                 # [B, 512]
    sxa = np.concatenate([sx, np.full((B, 1), float(N))], axis=1)  # [B, 513]
    m1row = sxa @ Rhat                                   # [B, 512]
    u1 = np.einsum('ij,bj->bi', Khat[:, :512], sx)       # [B, 513]

    aug2 = np.zeros((B, 2, GP + D), f)
    aug2[:, 0, :513] = u1.astype(f)
    aug2[:, 1, :513] = Khat[:, 512].astype(f)[None, :]
    aug2[:, 0, GP:GP + D] = Rhat[512].astype(f)[None, :]
    aug2[:, 1, GP:GP + D] = m1row.astype(f)

    # v = P_full[512] = Khat[512] @ G_aug @ Rhat is a pure function of
    # host-known data (three matvecs via associativity) — computed here
    # in f64 and added to the device output after the gather
    kvec = Khat[512]                                     # [513]
    w = np.einsum('bnd,d->bn', x.astype(np.float64), kvec[:512]) + kvec[512]
    u = np.concatenate([np.einsum('bnd,bn->bd', x.astype(np.float64), w),
                        w.sum(axis=1)[:, None]], axis=1)  # [B, 513]
    v_host = (u @ Rhat).astype(f)                        # [B, 512]

    # fp8 phase-1 layout, pair-batched:
    # xa8[b, pr, p, h, s, d] = x[b, (2*pr+s)*256 + h*128 + p, d]
    xa8 = np.ascontiguousarray(
        x.astype(F8).reshape(B, NT2 // 2, 2, 2, 128, D)
         .transpose(0, 1, 4, 3, 2, 5))

    # xat[b, t, p, c, j] = x[b, t*128+j, c*128+p] — per-tile [128, 4, 128]
    # lhsT blocks of x^T
    xat = np.ascontiguousarray(
        x.transpose(0, 2, 1)                     # [B, 512, 4096]
         .reshape(B, 4, 128, NT, 128)            # [B, c, p, t, j]
         .transpose(0, 3, 2, 1, 4)               # [B, t, p, c, j]
    ).astype(BF)

    cb_bf = cb.astype(BF)
    ident = np.eye(128, dtype=f).astype(BF)
    in_maps = [
        {"xa8": xa8[b], "xat": xat[b], "cb": cb_bf, "ident": ident,
         "aug2": aug2[b].astype(BF)}
        for b in range(B)
    ]
    return in_maps, v_host


def kernel(x, Wq1_w, Wq1_b, Wq2_w, Wq2_b, WR_w, WR_b):
    x = np.asarray(x, dtype=np.float32)
    args = [np.asarray(a, dtype=np.float32)
            for a in (Wq1_w, Wq1_b, Wq2_w, Wq2_b, WR_w, WR_b)]
    in_maps, v_host = _prep_host(x, *args)

    nc = _build()
    # the axon-tunneled device occasionally starts in a wedged state
    # (NRT_EXEC_UNIT_UNRECOVERABLE) and recovers on the next attempt
    last_err = None
    for attempt in range(3):
        try:
            res = run_bass_kernel_spmd(nc, in_maps, core_ids=list(range(N_CORES)))
            break
        except Exception as e:  # noqa: BLE001
            last_err = e
            import time as _time
            _time.sleep(2.0)
            try:
                import jax
                jax.clear_caches()
            except Exception:
                pass
    else:
        raise last_err
    return np.stack([res.results[b]["out"].astype(np.float32) + v_host[b]
                     for b in range(B)])
